# revision 1
# baseline (speedup 1.0000x reference)
"""Trainium2 Bass kernel for nn_MessagePassingGNN (8-core SPMD).

Strategy:
  - Sort edges (with self-loops) by target node; shard TARGET NODES across
    the 8 cores (6250 each) so each core owns a contiguous edge range and
    the segment-sum aggregation is core-local (no all-reduce).
  - Per layer, each core gathers source-node features from a replicated
    bf16 feature table in DRAM via dma_gather(transpose=True), which yields
    feature-major tiles that feed the message-MLP matmuls directly (no
    on-chip transposes). Target-side gathers read a core-local shard table
    so they never wait on the collective.
  - The scatter-mean aggregation runs on the tensor engine: a scaled one-hot
    matrix P[e, n] = (tgt_rel[e] == n) / count[tgt_e] is built by one fused
    DVE tensor_scalar per 128-edge tile, then agg += m3_tile.T @ P_tile
    accumulates in PSUM per 128-target-node block.
  - GRU update is node-sharded; updated shard features are AllGather'd into
    every core's table for the next layer. The decoder runs on the local
    shard; the host concatenates the 8 shards.

All matmuls are bf16 with fp32 PSUM accumulation; GRU elementwise math is
fp32. Host-measured end-to-end L2 relative error vs fp32 reference ~1e-2.
"""

import math

import numpy as np
import ml_dtypes

# Problem constants (hardcoded per harness contract).
N, IN_DIM, D, H, E, LAYERS = 50000, 16, 128, 256, 800000, 3
NCORES = 8
SH = N // NCORES            # 6250 nodes per shard
NB = (SH + 127) // 128      # 49 blocks of 128 target nodes
SHP = NB * 128              # 6272 padded shard width
SPLIT = 32768               # int16 index split for the gather table
BF16 = ml_dtypes.bfloat16

_PROGRAM_CACHE = {}
_RUN_KWARGS = {}       # test harness may set {"trace": True}
_LAST_RESULTS = None   # BassKernelResults of the most recent run


# ----------------------------------------------------------------------------
# Host-side preprocessing
# ----------------------------------------------------------------------------

def _wrap_idx(idx_i16):
    """dma_gather index layout: index i lives at [i % 16, i // 16],
    replicated across the 8 groups of 16 partitions."""
    n = idx_i16.shape[0]
    arr = idx_i16.reshape(n // 16, 16).T
    return np.tile(arr, (8, 1))


def _prep(x, edge_index):
    loops = np.arange(N, dtype=np.int64)
    src = np.concatenate([np.asarray(edge_index[0]), loops])
    tgt = np.concatenate([np.asarray(edge_index[1]), loops])
    order = np.argsort(tgt, kind="stable")
    src_s = src[order].astype(np.int32)
    tgt_s = tgt[order].astype(np.int32)
    counts = np.zeros(N, np.float32)
    np.add.at(counts, tgt_s, 1.0)
    cinv_n = (1.0 / counts).astype(np.float32)

    node_starts = np.searchsorted(tgt_s, np.arange(N + 1))
    lows = np.zeros((NCORES, NB), np.int64)
    highs = np.zeros((NCORES, NB), np.int64)
    rng = {}
    for c in range(NCORES):
        for b in range(NB):
            lo_node = c * SH + b * 128
            hi_node = min(c * SH + SH, lo_node + 128)
            e0, e1 = node_starts[lo_node], node_starts[hi_node]
            nl = int((src_s[e0:e1] < SPLIT).sum())
            lows[c, b] = nl
            highs[c, b] = (e1 - e0) - nl
            rng[(c, b)] = (e0, e1)
    Bl = [int(max(1, math.ceil(lows[:, b].max() / 128))) for b in range(NB)]
    Bh = [int(max(1, math.ceil(highs[:, b].max() / 128))) for b in range(NB)]

    meta = {"Bl": Bl, "Bh": Bh}
    ntiles = sum(Bl) + sum(Bh)
    nslots = ntiles * 128

    per_core = []
    for c in range(NCORES):
        idx_src = np.zeros(nslots, np.int16)
        idx_tgt = np.zeros(nslots, np.int16)
        tgt_rel = np.full(nslots, -1.0, np.float32)
        cinv_e = np.zeros(nslots, np.float32)
        off = 0
        for b in range(NB):
            e0, e1 = rng[(c, b)]
            s, t = src_s[e0:e1], tgt_s[e0:e1]
            lo = s < SPLIT
            for mask, cap, base in ((lo, Bl[b], 0), (~lo, Bh[b], SPLIT)):
                sh_, th_ = s[mask], t[mask]
                n = sh_.shape[0]
                idx_src[off:off + n] = (sh_ - base).astype(np.int16)
                idx_tgt[off:off + n] = (th_ - c * SH).astype(np.int16)
                tgt_rel[off:off + n] = (th_ - (c * SH + b * 128)).astype(np.float32)
                cinv_e[off:off + n] = cinv_n[th_]
                off += cap * 128
        assert off == nslots

        src_cols, tgt_cols = [], []
        off = 0
        for b in range(NB):
            wl, wh = Bl[b] * 128, Bh[b] * 128
            src_cols.append(_wrap_idx(idx_src[off:off + wl]))
            src_cols.append(_wrap_idx(idx_src[off + wl:off + wl + wh]))
            tgt_cols.append(_wrap_idx(idx_tgt[off:off + wl + wh]))
            off += wl + wh

        xs = np.zeros((IN_DIM, SHP), np.float32)
        xs[:, :SH] = np.asarray(x[c * SH:(c + 1) * SH]).T
        per_core.append({
            "x_sh_t": xs.astype(BF16),
            "idx_src": np.concatenate(src_cols, axis=1),
            "idx_tgt": np.concatenate(tgt_cols, axis=1),
            "tgt_rel": tgt_rel.reshape(ntiles, 128).T.copy(),
            "cinv_e": cinv_e.reshape(ntiles, 128).T.copy(),
        })
    return meta, per_core


def _prep_weights(inp):
    f32 = np.float32
    bf = lambda a: np.ascontiguousarray(np.asarray(a, f32)).astype(BF16)
    w = {}
    w["wenc"] = bf(inp["enc_W"])
    w["benc"] = np.asarray(inp["enc_b"], f32).reshape(128, 1).copy()
    w["w1t"] = bf(inp["msg_W1"][:, :D, :])
    w["w1s"] = bf(inp["msg_W1"][:, D:, :])
    w["w2a"] = bf(inp["msg_W2"][:, :128, :])
    w["w2b"] = bf(inp["msg_W2"][:, 128:, :])
    w["w3a"] = bf(inp["msg_W3"][:, :128, :])
    w["w3b"] = bf(inp["msg_W3"][:, 128:, :])
    w["b1"] = np.stack([np.asarray(b, f32).reshape(2, 128).T for b in inp["msg_b1"]])
    w["b2"] = np.stack([np.asarray(b, f32).reshape(2, 128).T for b in inp["msg_b2"]])
    w["wih"] = bf(inp["gru_Wih"])
    w["whh"] = bf(inp["gru_Whh"])
    bgi = np.stack([np.asarray(inp["msg_b3"][l], f32)
                    @ np.asarray(inp["gru_Wih"][l], f32)
                    + np.asarray(inp["gru_bih"][l], f32) for l in range(LAYERS)])
    bhh = np.asarray(inp["gru_bhh"], f32)
    w["brz"] = np.stack([(bgi[l, :2 * D] + bhh[l, :2 * D]).reshape(2, 128).T
                         for l in range(LAYERS)])
    w["bgin"] = np.stack([bgi[l, 2 * D:].reshape(128, 1) for l in range(LAYERS)])
    w["bghn"] = np.stack([bhh[l, 2 * D:].reshape(128, 1) for l in range(LAYERS)])
    w["wd1"] = bf(inp["dec_W1"])
    w["wd2a"] = bf(inp["dec_W2"][:128, :])
    w["wd2b"] = bf(inp["dec_W2"][128:, :])
    w["wd3a"] = bf(inp["dec_W3"][:128, :])
    w["wd3b"] = bf(inp["dec_W3"][128:, :])
    w["bd1"] = np.asarray(inp["dec_b1"], f32).reshape(2, 128).T.copy()
    w["bd2"] = np.asarray(inp["dec_b2"], f32).reshape(2, 128).T.copy()
    return w


# ----------------------------------------------------------------------------
# Bass program
# ----------------------------------------------------------------------------

def _build_program(meta, debug=False, repeat=1):
    import concourse.bacc as bacc
    import concourse.mybir as mybir
    import concourse.tile as tile
    from concourse import library_config
    from concourse.masks import make_identity

    Bl, Bh = meta["Bl"], meta["Bh"]
    ntiles = sum(Bl) + sum(Bh)
    nslots = ntiles * 128
    maxW = max((Bl[b] + Bh[b]) * 128 for b in range(NB))
    dt = mybir.dt
    AF = mybir.ActivationFunctionType
    OP = mybir.AluOpType

    nc = bacc.Bacc("TRN2", target_bir_lowering=False, debug=debug,
                   num_devices=NCORES)

    ext_in = lambda n, s, d: nc.dram_tensor(n, s, d, kind="ExternalInput")
    x_sh_t = ext_in("x_sh_t", [IN_DIM, SHP], dt.bfloat16)
    idx_src_d = ext_in("idx_src", [128, nslots // 16], dt.int16)
    idx_tgt_d = ext_in("idx_tgt", [128, nslots // 16], dt.int16)
    tgt_rel_d = ext_in("tgt_rel", [128, ntiles], dt.float32)
    cinv_d = ext_in("cinv_e", [128, ntiles], dt.float32)
    wenc_d = ext_in("wenc", [IN_DIM, 128], dt.bfloat16)
    benc_d = ext_in("benc", [128, 1], dt.float32)
    w1t_d = ext_in("w1t", [LAYERS, 128, H], dt.bfloat16)
    w1s_d = ext_in("w1s", [LAYERS, 128, H], dt.bfloat16)
    w2a_d = ext_in("w2a", [LAYERS, 128, H], dt.bfloat16)
    w2b_d = ext_in("w2b", [LAYERS, 128, H], dt.bfloat16)
    w3a_d = ext_in("w3a", [LAYERS, 128, D], dt.bfloat16)
    w3b_d = ext_in("w3b", [LAYERS, 128, D], dt.bfloat16)
    b1_d = ext_in("b1", [LAYERS, 128, 2], dt.float32)
    b2_d = ext_in("b2", [LAYERS, 128, 2], dt.float32)
    wih_d = ext_in("wih", [LAYERS, 128, 3 * D], dt.bfloat16)
    whh_d = ext_in("whh", [LAYERS, 128, 3 * D], dt.bfloat16)
    brz_d = ext_in("brz", [LAYERS, 128, 2], dt.float32)
    bgin_d = ext_in("bgin", [LAYERS, 128, 1], dt.float32)
    bghn_d = ext_in("bghn", [LAYERS, 128, 1], dt.float32)
    wd1_d = ext_in("wd1", [128, H], dt.bfloat16)
    wd2a_d = ext_in("wd2a", [128, H], dt.bfloat16)
    wd2b_d = ext_in("wd2b", [128, H], dt.bfloat16)
    wd3a_d = ext_in("wd3a", [128, 1], dt.bfloat16)
    wd3b_d = ext_in("wd3b", [128, 1], dt.bfloat16)
    bd1_d = ext_in("bd1", [128, 2], dt.float32)
    bd2_d = ext_in("bd2", [128, 2], dt.float32)
    y_d = nc.dram_tensor("y", [1, SH], dt.float32, kind="ExternalOutput")

    table = nc.dram_tensor("table", [N, D], dt.bfloat16, addr_space="Shared")
    loc_tab = nc.dram_tensor("loc_tab", [SHP, D], dt.bfloat16)
    cc_in = nc.dram_tensor("cc_in", [SH, D], dt.bfloat16)

    groups512 = lambda W: [(g0, min(512, W - g0)) for g0 in range(0, W, 512)]
    as3d = lambda ap: ap.rearrange("p (o n) -> p o n", o=1)

    with tile.TileContext(nc, num_cores=NCORES) as tc:
        nc.gpsimd.load_library(library_config.mlp)

        with (
            tc.tile_pool(name="const", bufs=1) as cpool,
            tc.tile_pool(name="state", bufs=1) as spool,
            tc.tile_pool(name="gather", bufs=2) as gpool,
            tc.tile_pool(name="mlp", bufs=2) as mpool,
            tc.tile_pool(name="psA", bufs=1, space="PSUM") as ppA,
            tc.tile_pool(name="psB", bufs=1, space="PSUM") as ppB,
            tc.tile_pool(name="psC", bufs=2, space="PSUM") as ppC,
        ):
            def ld(dram_ap, nm):
                t = cpool.tile(list(dram_ap.shape), dram_ap.dtype, tag=nm)
                nc.sync.dma_start(out=t[:], in_=dram_ap)
                return t

            idx_src = ld(idx_src_d.ap(), "idx_src")
            idx_tgt = ld(idx_tgt_d.ap(), "idx_tgt")
            tgt_rel = ld(tgt_rel_d.ap(), "tgt_rel")
            cinv = ld(cinv_d.ap(), "cinv")
            xsh = ld(x_sh_t.ap(), "xsh")
            wenc = ld(wenc_d.ap(), "wenc")
            benc = ld(benc_d.ap(), "benc")
            w1t = [ld(w1t_d.ap()[l], f"w1t{l}") for l in range(LAYERS)]
            w1s = [ld(w1s_d.ap()[l], f"w1s{l}") for l in range(LAYERS)]
            w2a = [ld(w2a_d.ap()[l], f"w2a{l}") for l in range(LAYERS)]
            w2b = [ld(w2b_d.ap()[l], f"w2b{l}") for l in range(LAYERS)]
            w3a = [ld(w3a_d.ap()[l], f"w3a{l}") for l in range(LAYERS)]
            w3b = [ld(w3b_d.ap()[l], f"w3b{l}") for l in range(LAYERS)]
            b1 = [ld(b1_d.ap()[l], f"b1{l}") for l in range(LAYERS)]
            b2 = [ld(b2_d.ap()[l], f"b2{l}") for l in range(LAYERS)]
            wih = [ld(wih_d.ap()[l], f"wih{l}") for l in range(LAYERS)]
            whh = [ld(whh_d.ap()[l], f"whh{l}") for l in range(LAYERS)]
            brz = [ld(brz_d.ap()[l], f"brz{l}") for l in range(LAYERS)]
            bgin = [ld(bgin_d.ap()[l], f"bgin{l}") for l in range(LAYERS)]
            bghn = [ld(bghn_d.ap()[l], f"bghn{l}") for l in range(LAYERS)]
            wd1 = ld(wd1_d.ap(), "wd1")
            wd2a = ld(wd2a_d.ap(), "wd2a")
            wd2b = ld(wd2b_d.ap(), "wd2b")
            wd3a = ld(wd3a_d.ap(), "wd3a")
            wd3b = ld(wd3b_d.ap(), "wd3b")
            bd1 = ld(bd1_d.ap(), "bd1")
            bd2 = ld(bd2_d.ap(), "bd2")

            iota = cpool.tile([128, 128], dt.float32, tag="iota")
            nc.gpsimd.iota(iota[:], pattern=[[1, 128]], base=0,
                           channel_multiplier=0,
                           allow_small_or_imprecise_dtypes=True)
            ident = cpool.tile([128, 128], dt.bfloat16, tag="ident")
            make_identity(nc, ident[:])

            h_f32 = spool.tile([128, SHP], dt.float32, tag="h_f32")
            h_bf = spool.tile([128, SHP], dt.bfloat16, tag="h_bf")
            h_nm = spool.tile([128, SHP], dt.bfloat16, tag="h_nm")
            agg_bf = spool.tile([128, SHP], dt.bfloat16, tag="agg_bf")

            def finish_layer():
                for b in range(NB):
                    tp = ppB.tile([128, 128], dt.bfloat16, tag="m3p")
                    nc.tensor.transpose(tp[:], h_bf[:, b * 128:(b + 1) * 128],
                                        ident[:])
                    nc.vector.tensor_copy(h_nm[:, b * 128:(b + 1) * 128], tp[:])
                nc.sync.dma_start(
                    out=loc_tab.ap().rearrange("(b p) d -> p b d", p=128),
                    in_=h_nm[:].rearrange("p (b d) -> p b d", d=D))
                nbf = SH // 128  # full 128-node blocks in the shard
                nc.sync.dma_start(
                    out=cc_in.ap()[:nbf * 128].rearrange("(b p) d -> p b d", p=128),
                    in_=h_nm[:, :nbf * 128].rearrange("p (b d) -> p b d", d=D))
                if SH > nbf * 128:
                    nc.sync.dma_start(
                        out=cc_in.ap()[nbf * 128:SH],
                        in_=h_nm[:SH - nbf * 128, nbf * 128:(nbf + 1) * 128])
                nc.gpsimd.collective_compute(
                    "AllGather", OP.bypass,
                    replica_groups=[list(range(NCORES))],
                    ins=[cc_in.ap()], outs=[table.ap()])

            # ---------------- encoder ----------------
            for n0, w in groups512(SHP):
                ps = ppA.tile([128, 512], dt.float32, tag="mp0")
                nc.tensor.matmul(ps[:, :w], lhsT=wenc[:], rhs=xsh[:, n0:n0 + w],
                                 start=True, stop=True)
                nc.scalar.activation(h_f32[:, n0:n0 + w], ps[:, :w], AF.Tanh,
                                     bias=benc[:, 0:1])
                nc.vector.tensor_copy(h_bf[:, n0:n0 + w], h_f32[:, n0:n0 + w])
            finish_layer()

            # ---------------- message-passing layers ----------------
            # repeat>1 re-runs the layer stack for timing (garbage numerics
            # after the first pass; used only by the benchmark).
            for l in [l for _ in range(repeat) for l in range(LAYERS)]:
                tile_idx = 0
                slot_off = 0
                for b in range(NB):
                    wl, wh = Bl[b] * 128, Bh[b] * 128
                    W = wl + wh
                    gsrc = gpool.tile([128, maxW], dt.bfloat16, tag="gsrc")
                    gtgt = gpool.tile([128, maxW], dt.bfloat16, tag="gtgt")
                    nc.gpsimd.dma_gather(
                        as3d(gsrc[:, 0:wl]), table.ap()[0:SPLIT],
                        idx_src[:, slot_off:slot_off + wl // 16],
                        wl, wl, D, transpose=True, single_packet=False)
                    nc.gpsimd.dma_gather(
                        as3d(gsrc[:, wl:W]), table.ap()[SPLIT:N],
                        idx_src[:, slot_off + wl // 16:slot_off + W // 16],
                        wh, wh, D, transpose=True, single_packet=False)
                    nc.gpsimd.dma_gather(
                        as3d(gtgt[:, 0:W]), loc_tab.ap(),
                        idx_tgt[:, slot_off:slot_off + W // 16],
                        W, W, D, transpose=True, single_packet=False)
                    slot_off += W // 16

                    aggp = ppC.tile([128, 128], dt.float32, tag="aggp")
                    first_tile = 0
                    for g0 in range(0, W, 1024):
                        w = min(1024, W - g0)
                        nt = w // 128
                        halves = [(h0, min(512, w - h0))
                                  for h0 in range(0, w, 512)]
                        m1s, m2s = [], []
                        for ci in range(2):
                            cs = slice(ci * 128, (ci + 1) * 128)
                            p = ppA.tile([128, 1024], dt.float32, tag=f"mp{ci}")
                            for h0, hw in halves:
                                nc.tensor.matmul(
                                    p[:, h0:h0 + hw], lhsT=w1t[l][:, cs],
                                    rhs=gtgt[:, g0 + h0:g0 + h0 + hw],
                                    start=True, stop=False)
                            for h0, hw in halves:
                                nc.tensor.matmul(
                                    p[:, h0:h0 + hw], lhsT=w1s[l][:, cs],
                                    rhs=gsrc[:, g0 + h0:g0 + h0 + hw],
                                    start=False, stop=True)
                            s = mpool.tile([128, 1024], dt.bfloat16,
                                           tag=f"m1s{ci}")
                            nc.scalar.activation(s[:, :w], p[:, :w], AF.Tanh,
                                                 bias=b1[l][:, ci:ci + 1])
                            m1s.append(s)
                        for ci in range(2):
                            cs = slice(ci * 128, (ci + 1) * 128)
                            p = ppA.tile([128, 1024], dt.float32, tag=f"mp{ci}")
                            for h0, hw in halves:
                                nc.tensor.matmul(
                                    p[:, h0:h0 + hw], lhsT=w2a[l][:, cs],
                                    rhs=m1s[0][:, h0:h0 + hw],
                                    start=True, stop=False)
                            for h0, hw in halves:
                                nc.tensor.matmul(
                                    p[:, h0:h0 + hw], lhsT=w2b[l][:, cs],
                                    rhs=m1s[1][:, h0:h0 + hw],
                                    start=False, stop=True)
                            s = mpool.tile([128, 1024], dt.bfloat16,
                                           tag=f"m2s{ci}")
                            nc.scalar.activation(s[:, :w], p[:, :w], AF.Tanh,
                                                 bias=b2[l][:, ci:ci + 1])
                            m2s.append(s)
                        m3p = ppB.tile([128, 1024], dt.float32, tag="m3p")
                        for t in range(nt):
                            ts = slice(t * 128, (t + 1) * 128)
                            nc.tensor.matmul(m3p[:, ts], lhsT=m2s[0][:, ts],
                                             rhs=w3a[l][:], start=True, stop=False)
                            nc.tensor.matmul(m3p[:, ts], lhsT=m2s[1][:, ts],
                                             rhs=w3b[l][:], start=False, stop=True)
                        m3s = mpool.tile([128, 1024], dt.bfloat16, tag="m3s")
                        nc.vector.tensor_copy(m3s[:, :w], m3p[:, :w])
                        psel = mpool.tile([128, 1024], dt.bfloat16, tag="psel")
                        for t in range(nt):
                            col = tile_idx + first_tile + t
                            nc.vector.tensor_scalar(
                                psel[:, t * 128:(t + 1) * 128], iota[:],
                                tgt_rel[:, col:col + 1], cinv[:, col:col + 1],
                                OP.is_equal, OP.mult)
                        for t in range(nt):
                            ts = slice(t * 128, (t + 1) * 128)
                            nc.tensor.matmul(
                                aggp[:], lhsT=m3s[:, ts], rhs=psel[:, ts],
                                start=(first_tile + t == 0),
                                stop=(first_tile + t == W // 128 - 1))
                        first_tile += nt
                    tile_idx += W // 128
                    nc.vector.tensor_copy(agg_bf[:, b * 128:(b + 1) * 128],
                                          aggp[:])

                # ---- GRU update over the node shard ----
                for n0, w in groups512(SHP):
                    ns = slice(n0, n0 + w)
                    rz = []
                    for k in range(2):
                        ks = slice(k * 128, (k + 1) * 128)
                        p = ppA.tile([128, 512], dt.float32, tag=f"mp{k}")
                        nc.tensor.matmul(p[:, :w], lhsT=wih[l][:, ks],
                                         rhs=agg_bf[:, ns], start=True, stop=False)
                        nc.tensor.matmul(p[:, :w], lhsT=whh[l][:, ks],
                                         rhs=h_bf[:, ns], start=False, stop=True)
                        s = mpool.tile([128, 512], dt.bfloat16, tag=f"m1s{k}")
                        nc.scalar.activation(s[:, :w], p[:, :w], AF.Sigmoid,
                                             bias=brz[l][:, k:k + 1])
                        rz.append(s)
                    gin = ppB.tile([128, 512], dt.float32, tag="m3p")
                    nc.tensor.matmul(gin[:, :w], lhsT=wih[l][:, 2 * 128:],
                                     rhs=agg_bf[:, ns], start=True, stop=True)
                    ghn = ppA.tile([128, 512], dt.float32, tag="mp0")
                    nc.tensor.matmul(ghn[:, :w], lhsT=whh[l][:, 2 * 128:],
                                     rhs=h_bf[:, ns], start=True, stop=True)
                    ghnb = mpool.tile([128, 512], dt.float32, tag="m2s0")
                    nc.vector.tensor_scalar(ghnb[:, :w], ghn[:, :w],
                                            bghn[l][:, 0:1], None, OP.add)
                    t1 = mpool.tile([128, 512], dt.float32, tag="m2s1")
                    nc.vector.tensor_tensor(t1[:, :w], rz[0][:, :w], ghnb[:, :w],
                                            OP.mult)
                    pre = mpool.tile([128, 512], dt.float32, tag="m3s")
                    nc.vector.tensor_tensor(pre[:, :w], gin[:, :w], t1[:, :w],
                                            OP.add)
                    nn = mpool.tile([128, 512], dt.float32, tag="psel")
                    nc.scalar.activation(nn[:, :w], pre[:, :w], AF.Tanh,
                                         bias=bgin[l][:, 0:1])
                    dd = mpool.tile([128, 512], dt.float32, tag="dd")
                    nc.vector.tensor_tensor(dd[:, :w], h_f32[:, ns], nn[:, :w],
                                            OP.subtract)
                    ee = mpool.tile([128, 512], dt.float32, tag="ee")
                    nc.vector.tensor_tensor(ee[:, :w], rz[1][:, :w], dd[:, :w],
                                            OP.mult)
                    nc.vector.tensor_tensor(h_f32[:, ns], nn[:, :w], ee[:, :w],
                                            OP.add)
                    nc.vector.tensor_copy(h_bf[:, ns], h_f32[:, ns])

                if l < LAYERS - 1:
                    finish_layer()

            # ---------------- decoder ----------------
            for n0, w in groups512(SHP):
                ns = slice(n0, n0 + w)
                o1s, o2s = [], []
                for ci in range(2):
                    cs = slice(ci * 128, (ci + 1) * 128)
                    p = ppA.tile([128, 512], dt.float32, tag=f"mp{ci}")
                    nc.tensor.matmul(p[:, :w], lhsT=wd1[:, cs],
                                     rhs=h_bf[:, ns], start=True, stop=True)
                    s = mpool.tile([128, 512], dt.bfloat16, tag=f"m1s{ci}")
                    nc.scalar.activation(s[:, :w], p[:, :w], AF.Tanh,
                                         bias=bd1[:, ci:ci + 1])
                    o1s.append(s)
                for ci in range(2):
                    cs = slice(ci * 128, (ci + 1) * 128)
                    p = ppA.tile([128, 512], dt.float32, tag=f"mp{ci}")
                    nc.tensor.matmul(p[:, :w], lhsT=wd2a[:, cs],
                                     rhs=o1s[0][:, :w], start=True, stop=False)
                    nc.tensor.matmul(p[:, :w], lhsT=wd2b[:, cs],
                                     rhs=o1s[1][:, :w], start=False, stop=True)
                    s = mpool.tile([128, 512], dt.bfloat16, tag=f"m2s{ci}")
                    nc.scalar.activation(s[:, :w], p[:, :w], AF.Tanh,
                                         bias=bd2[:, ci:ci + 1])
                    o2s.append(s)
                o3p = ppB.tile([1, 512], dt.float32, tag="m3p")
                nc.tensor.matmul(o3p[:, :w], lhsT=wd3a[:], rhs=o2s[0][:, :w],
                                 start=True, stop=False)
                nc.tensor.matmul(o3p[:, :w], lhsT=wd3b[:], rhs=o2s[1][:, :w],
                                 start=False, stop=True)
                yt = mpool.tile([1, 512], dt.float32, tag="m3s")
                nc.scalar.copy(yt[:, :w], o3p[:, :w])
                we = min(w, SH - n0) if n0 < SH else 0
                if we > 0:
                    nc.sync.dma_start(out=y_d.ap()[:, n0:n0 + we],
                                      in_=yt[:, :we])

    nc.compile()
    return nc


# ----------------------------------------------------------------------------
# Entry point
# ----------------------------------------------------------------------------

def kernel(**inputs) -> np.ndarray:
    from concourse.bass_utils import run_bass_kernel_spmd

    meta, per_core = _prep(np.asarray(inputs["x"], np.float32),
                           np.asarray(inputs["edge_index"]))
    w = _prep_weights(inputs)

    key = (tuple(meta["Bl"]), tuple(meta["Bh"]))
    if key not in _PROGRAM_CACHE:
        _PROGRAM_CACHE[key] = _build_program(meta)
    nc = _PROGRAM_CACHE[key]

    in_maps = []
    for c in range(NCORES):
        m = dict(per_core[c])
        m.update(w)
        in_maps.append(m)
    res = run_bass_kernel_spmd(nc, in_maps, core_ids=list(range(NCORES)),
                               **_RUN_KWARGS)
    global _LAST_RESULTS
    _LAST_RESULTS = res
    out = np.concatenate([res.results[c]["y"][0] for c in range(NCORES)])
    return (out + np.asarray(inputs["dec_b3"], np.float32)[0]).astype(np.float32)



# revision 14
# speedup vs baseline: 36.0179x; 36.0179x over previous
"""Trainium2 Bass kernel for nn_MessagePassingGNN (8-core SPMD).

Strategy:
  - Sort edges (with self-loops) by target node; shard TARGET NODES across
    the 8 cores (6250 each) so each core owns a contiguous edge range and
    the segment-sum aggregation is core-local (no all-reduce).
  - Per layer, each core gathers source-node features from a replicated
    bf16 feature table in DRAM via dma_gather(transpose=True), which yields
    feature-major tiles that feed the message-MLP matmuls directly (no
    on-chip transposes). Target-side gathers read a core-local shard table
    so they never wait on the collective.
  - The scatter-mean aggregation runs on the tensor engine: a scaled one-hot
    matrix P[e, n] = (tgt_rel[e] == n) / count[tgt_e] is built by one fused
    DVE tensor_scalar per 128-edge tile, then agg += m3_tile.T @ P_tile
    accumulates in PSUM per 128-target-node block.
  - GRU update is node-sharded; updated shard features are AllGather'd into
    every core's table for the next layer. The decoder runs on the local
    shard; the host concatenates the 8 shards.

Host<->device transfer is minimized (it dominates wall-clock through the
axon tunnel):
  - dma_gather index tables are staged compactly as [16, W/16] and
    replicated across the 8 partition groups on-device (8x fewer bytes).
  - Per-edge target-relative ids and segment counts ship as int8 and are
    converted / reciprocated on-device.
  - All weights/biases ship as two flat blobs, each sharded 1/8th per core,
    and are reassembled on-device with an AllGather collective (8x fewer
    bytes than replicating them).
  - The PJRT executable for the Bass program is built once and cached, so
    repeat calls pay only input staging + device execution (this matches
    what run_bass_kernel_spmd does under axon, minus the per-call
    jax.jit/shard_map rebuild).

All matmuls are bf16 with fp32 PSUM accumulation; GRU elementwise math is
fp32. Host-measured end-to-end L2 relative error vs fp32 reference ~1e-2.
"""

import math

import numpy as np
import ml_dtypes

# Problem constants (hardcoded per harness contract).
N, IN_DIM, D, H, E, LAYERS = 50000, 16, 128, 256, 800000, 3
NCORES = 8
SH = N // NCORES            # 6250 nodes per shard
NB = (SH + 127) // 128      # 49 blocks of 128 target nodes
SHP = NB * 128              # 6272 padded shard width
SPLIT = 32768               # int16 index split for the gather table
BF16 = ml_dtypes.bfloat16

_PROGRAM_CACHE = {}


# ----------------------------------------------------------------------------
# Weight blob layout (static; shared by host packer and device program)
# ----------------------------------------------------------------------------

def _blob_layout():
    L16 = [("wenc", (IN_DIM, 128))]
    for l in range(LAYERS):
        L16 += [(f"w1t{l}", (128, H)), (f"w1s{l}", (128, H)),
                (f"w2a{l}", (128, H)), (f"w2b{l}", (128, H)),
                (f"w3a{l}", (128, D)), (f"w3b{l}", (128, D)),
                (f"wih{l}", (128, 3 * D)), (f"whh{l}", (128, 3 * D))]
    L16 += [("wd1", (128, H)), ("wd2a", (128, H)), ("wd2b", (128, H)),
            ("wd3a", (128, 1)), ("wd3b", (128, 1))]
    L32 = [("benc", (128, 1))]
    for l in range(LAYERS):
        L32 += [(f"b1{l}", (128, 2)), (f"b2{l}", (128, 2)),
                (f"brz{l}", (128, 2)), (f"bgin{l}", (128, 1)),
                (f"bghn{l}", (128, 1))]
    L32 += [("bd1", (128, 2)), ("bd2", (128, 2))]

    def offsets(items):
        offs, o = {}, 0
        for name, shp in items:
            offs[name] = (o, shp)
            o += shp[0] * shp[1]
        return offs, o + ((-o) % (NCORES * 128))

    O16, T16 = offsets(L16)
    O32, T32 = offsets(L32)
    return O16, T16, O32, T32


OFF16, TOT16, OFF32, TOT32 = _blob_layout()


# ----------------------------------------------------------------------------
# Host-side preprocessing
# ----------------------------------------------------------------------------

def _wrap16(idx_i16):
    """dma_gather index layout: index i lives at [i % 16, i // 16]. The
    8x partition-group replication happens on-device."""
    n = idx_i16.shape[0]
    return idx_i16.reshape(n // 16, 16).T


def _prep(x, edge_index):
    loops = np.arange(N, dtype=np.int64)
    src = np.concatenate([np.asarray(edge_index[0]), loops])
    tgt = np.concatenate([np.asarray(edge_index[1]), loops])
    order = np.argsort(tgt, kind="stable")
    src_s = src[order].astype(np.int32)
    tgt_s = tgt[order].astype(np.int32)
    counts = np.zeros(N, np.int32)
    np.add.at(counts, tgt_s, 1)
    assert counts.max() < 128, "int8 staging assumes max degree < 128"

    node_starts = np.searchsorted(tgt_s, np.arange(N + 1))
    lows = np.zeros((NCORES, NB), np.int64)
    highs = np.zeros((NCORES, NB), np.int64)
    rng = {}
    for c in range(NCORES):
        for b in range(NB):
            lo_node = c * SH + b * 128
            hi_node = min(c * SH + SH, lo_node + 128)
            e0, e1 = node_starts[lo_node], node_starts[hi_node]
            nl = int((src_s[e0:e1] < SPLIT).sum())
            lows[c, b] = nl
            highs[c, b] = (e1 - e0) - nl
            rng[(c, b)] = (e0, e1)
    Bl = [int(max(1, math.ceil(lows[:, b].max() / 128))) for b in range(NB)]
    Bh = [int(max(1, math.ceil(highs[:, b].max() / 128))) for b in range(NB)]

    meta = {"Bl": Bl, "Bh": Bh}
    ntiles = sum(Bl) + sum(Bh)
    nslots = ntiles * 128

    per_core = []
    for c in range(NCORES):
        idx_src = np.zeros(nslots, np.int16)
        tgt_rel = np.full(nslots, -1, np.int8)
        cnt_e = np.ones(nslots, np.int8)
        off = 0
        for b in range(NB):
            e0, e1 = rng[(c, b)]
            s, t = src_s[e0:e1], tgt_s[e0:e1]
            lo = s < SPLIT
            for mask, cap, base in ((lo, Bl[b], 0), (~lo, Bh[b], SPLIT)):
                sh_, th_ = s[mask], t[mask]
                n = sh_.shape[0]
                idx_src[off:off + n] = (sh_ - base).astype(np.int16)
                tgt_rel[off:off + n] = (th_ - (c * SH + b * 128)).astype(np.int8)
                cnt_e[off:off + n] = counts[th_].astype(np.int8)
                off += cap * 128
        assert off == nslots

        src_cols = []
        off = 0
        for b in range(NB):
            wl, wh = Bl[b] * 128, Bh[b] * 128
            src_cols.append(_wrap16(idx_src[off:off + wl]))
            src_cols.append(_wrap16(idx_src[off + wl:off + wl + wh]))
            off += wl + wh

        xs = np.zeros((IN_DIM, SHP), np.float32)
        xs[:, :SH] = np.asarray(x[c * SH:(c + 1) * SH]).T
        per_core.append({
            "x_sh_t": xs.astype(BF16),
            "idx16": np.concatenate(src_cols, axis=1),
            "aux8": np.concatenate(
                [_wrap16(tgt_rel), _wrap16(cnt_e)], axis=1).copy(),
        })
    return meta, per_core


def _prep_weights(inp):
    f32 = np.float32
    bf = lambda a: np.ascontiguousarray(np.asarray(a, f32)).astype(BF16)
    w = {}
    w["wenc"] = bf(inp["enc_W"])
    w["benc"] = np.asarray(inp["enc_b"], f32).reshape(128, 1)
    for l in range(LAYERS):
        w[f"w1t{l}"] = bf(inp["msg_W1"][l][:D, :])
        w[f"w1s{l}"] = bf(inp["msg_W1"][l][D:, :])
        w[f"w2a{l}"] = bf(inp["msg_W2"][l][:128, :])
        w[f"w2b{l}"] = bf(inp["msg_W2"][l][128:, :])
        w[f"w3a{l}"] = bf(inp["msg_W3"][l][:128, :])
        w[f"w3b{l}"] = bf(inp["msg_W3"][l][128:, :])
        w[f"b1{l}"] = np.asarray(inp["msg_b1"][l], f32).reshape(2, 128).T
        w[f"b2{l}"] = np.asarray(inp["msg_b2"][l], f32).reshape(2, 128).T
        w[f"wih{l}"] = bf(inp["gru_Wih"][l])
        w[f"whh{l}"] = bf(inp["gru_Whh"][l])
        bgi = (np.asarray(inp["msg_b3"][l], f32)
               @ np.asarray(inp["gru_Wih"][l], f32)
               + np.asarray(inp["gru_bih"][l], f32))
        bhh = np.asarray(inp["gru_bhh"][l], f32)
        w[f"brz{l}"] = (bgi[:2 * D] + bhh[:2 * D]).reshape(2, 128).T
        w[f"bgin{l}"] = bgi[2 * D:].reshape(128, 1)
        w[f"bghn{l}"] = bhh[2 * D:].reshape(128, 1)
    w["wd1"] = bf(inp["dec_W1"])
    w["wd2a"] = bf(inp["dec_W2"][:128, :])
    w["wd2b"] = bf(inp["dec_W2"][128:, :])
    w["wd3a"] = bf(inp["dec_W3"][:128, :])
    w["wd3b"] = bf(inp["dec_W3"][128:, :])
    w["bd1"] = np.asarray(inp["dec_b1"], f32).reshape(2, 128).T
    w["bd2"] = np.asarray(inp["dec_b2"], f32).reshape(2, 128).T

    blob16 = np.zeros(TOT16, BF16)
    for name, (off, shp) in OFF16.items():
        blob16[off:off + shp[0] * shp[1]] = w[name].reshape(-1)
    blob32 = np.zeros(TOT32, f32)
    for name, (off, shp) in OFF32.items():
        blob32[off:off + shp[0] * shp[1]] = w[name].reshape(-1)
    return blob16, blob32


# ----------------------------------------------------------------------------
# Bass program
# ----------------------------------------------------------------------------

def _build_program(meta, debug=False, repeat=1):
    import concourse.bacc as bacc
    import concourse.mybir as mybir
    import concourse.tile as tile
    from concourse import library_config
    from concourse.masks import make_identity
    from concourse.tile_rust import add_dep_helper

    Bl, Bh = meta["Bl"], meta["Bh"]
    ntiles = sum(Bl) + sum(Bh)
    nslots = ntiles * 128
    T = nslots // 16            # wrapped-layout column count
    maxW = max((Bl[b] + Bh[b]) * 128 for b in range(NB))
    dt = mybir.dt
    AF = mybir.ActivationFunctionType
    OP = mybir.AluOpType

    nc = bacc.Bacc("TRN2", target_bir_lowering=False, debug=debug,
                   num_devices=NCORES)

    ext_in = lambda n, s, d: nc.dram_tensor(n, s, d, kind="ExternalInput")
    x_sh_t = ext_in("x_sh_t", [IN_DIM, SHP], dt.bfloat16)
    idx16_d = ext_in("idx16", [16, T], dt.int16)
    aux8_d = ext_in("aux8", [16, 2 * T], dt.int8)
    wsh16_d = ext_in("wsh16", [1, TOT16 // NCORES], dt.bfloat16)
    wsh32_d = ext_in("wsh32", [1, TOT32 // NCORES], dt.float32)
    y_d = nc.dram_tensor("y", [1, SH], dt.float32, kind="ExternalOutput")

    wtmp16 = nc.dram_tensor("wtmp16", [1, TOT16 // NCORES], dt.bfloat16)
    wtmp32 = nc.dram_tensor("wtmp32", [1, TOT32 // NCORES], dt.float32)
    wfull16 = nc.dram_tensor("wfull16", [1, TOT16], dt.bfloat16,
                             addr_space="Shared")
    wfull32 = nc.dram_tensor("wfull32", [1, TOT32], dt.float32,
                             addr_space="Shared")
    table = nc.dram_tensor("table", [N, D], dt.bfloat16, addr_space="Shared")
    loc_tab = nc.dram_tensor("loc_tab", [SHP, D], dt.bfloat16)
    cc_in = nc.dram_tensor("cc_in", [SH, D], dt.bfloat16)

    groups512 = lambda W: [(g0, min(512, W - g0)) for g0 in range(0, W, 512)]
    as3d = lambda ap: ap.rearrange("p (o n) -> p o n", o=1)

    with tile.TileContext(nc, num_cores=NCORES) as tc:
        nc.gpsimd.load_library(library_config.mlp)

        with (
            tc.tile_pool(name="const", bufs=1) as cpool,
            tc.tile_pool(name="state", bufs=1) as spool,
            tc.tile_pool(name="gather", bufs=2) as gpool,
            tc.tile_pool(name="mlp", bufs=2) as mpool,
            tc.tile_pool(name="psA", bufs=1, space="PSUM") as ppA,
            tc.tile_pool(name="psB", bufs=1, space="PSUM") as ppB,
            tc.tile_pool(name="psC", bufs=2, space="PSUM") as ppC,
        ):
            # -------- reassemble the replicated weight blobs on-device ----
            # (collectives can't read IO tensors; bounce through internal DRAM)
            ld16 = nc.sync.dma_start(out=wtmp16.ap(), in_=wsh16_d.ap())
            ld32 = nc.sync.dma_start(out=wtmp32.ap(), in_=wsh32_d.ap())
            cc16 = nc.gpsimd.collective_compute(
                "AllGather", OP.bypass,
                replica_groups=[list(range(NCORES))],
                ins=[wtmp16.ap()], outs=[wfull16.ap()])
            cc32 = nc.gpsimd.collective_compute(
                "AllGather", OP.bypass,
                replica_groups=[list(range(NCORES))],
                ins=[wtmp32.ap()], outs=[wfull32.ap()])
            # DRAM RAW hazards aren't tracked by tile's shadow memory
            # (SBUF/PSUM only) — declare the edges explicitly.
            add_dep_helper(cc16.ins, ld16.ins, reason="allgather after stage")
            add_dep_helper(cc32.ins, ld32.ins, reason="allgather after stage")

            def ldw(nm):
                off, (p, w) = OFF16[nm]
                t = cpool.tile([p, w], dt.bfloat16, tag=nm)
                di = nc.sync.dma_start(
                    out=t[:],
                    in_=wfull16.ap()[0, off:off + p * w]
                        .rearrange("(p w) -> p w", w=w))
                add_dep_helper(di.ins, cc16.ins, reason="load after allgather")
                return t

            def ldb(nm):
                off, (p, w) = OFF32[nm]
                t = cpool.tile([p, w], dt.float32, tag=nm)
                di = nc.sync.dma_start(
                    out=t[:],
                    in_=wfull32.ap()[0, off:off + p * w]
                        .rearrange("(p w) -> p w", w=w))
                add_dep_helper(di.ins, cc32.ins, reason="load after allgather")
                return t

            def ld(dram_ap, nm, dtype=None):
                t = cpool.tile(list(dram_ap.shape), dtype or dram_ap.dtype,
                               tag=nm)
                nc.sync.dma_start(out=t[:], in_=dram_ap)
                return t

            xsh = ld(x_sh_t.ap(), "xsh")

            # replicate the compact [16, T] gather-index / tgt-rel tables
            # across the 8 partition groups (dma_gather wants them in all)
            idxrep = cpool.tile([128, T], dt.int16, tag="idxrep")
            trw_rep = cpool.tile([128, T], dt.int8, tag="trw_rep")
            for k in range(8):
                nc.sync.dma_start(out=idxrep[16 * k:16 * (k + 1), :],
                                  in_=idx16_d.ap())
                nc.sync.dma_start(out=trw_rep[16 * k:16 * (k + 1), :],
                                  in_=aux8_d.ap()[:, 0:T])

            # int16 target-gather indices: clamp(tgt_rel, 0, 127) + 128*block.
            # The clamp sends the pad slots (-1, or 255 if the int8 widen is
            # unsigned) to a real in-range node; the psel mask below still
            # sees the raw pad value and zeroes their contribution.
            idxrep_tgt = cpool.tile([128, T], dt.int16, tag="idxrep_tgt")
            nc.vector.tensor_copy(idxrep_tgt[:], trw_rep[:])
            nc.vector.tensor_scalar(idxrep_tgt[:], idxrep_tgt[:], 0, 127,
                                    OP.max, OP.min)
            c0 = 0
            for b in range(NB):
                c1 = c0 + (Bl[b] + Bh[b]) * 8
                if b:
                    nc.vector.tensor_scalar(idxrep_tgt[:, c0:c1],
                                            idxrep_tgt[:, c0:c1],
                                            128 * b, None, OP.add)
                c0 = c1

            # per-edge-slot tgt_rel / count in partitioned [e%128, e//128]
            # layout: wrapped (p, 8c+k) -> partitioned (16k+p, c)
            tr8p = cpool.tile([128, ntiles], dt.int8, tag="tr8p")
            cnt8p = cpool.tile([128, ntiles], dt.int8, tag="cnt8p")
            for k in range(8):
                nc.sync.dma_start(out=tr8p[16 * k:16 * (k + 1), :],
                                  in_=aux8_d.ap()[:, k:T:8])
                nc.sync.dma_start(out=cnt8p[16 * k:16 * (k + 1), :],
                                  in_=aux8_d.ap()[:, T + k:2 * T:8])
            tgt_rel = cpool.tile([128, ntiles], dt.float32, tag="tgt_rel")
            nc.vector.tensor_copy(tgt_rel[:], tr8p[:])
            cntf = cpool.tile([128, ntiles], dt.float32, tag="cntf")
            nc.vector.tensor_copy(cntf[:], cnt8p[:])
            cinv = cpool.tile([128, ntiles], dt.float32, tag="cinv")
            nc.vector.reciprocal(cinv[:], cntf[:])

            wenc = ldw("wenc")
            benc = ldb("benc")
            w1t = [ldw(f"w1t{l}") for l in range(LAYERS)]
            w1s = [ldw(f"w1s{l}") for l in range(LAYERS)]
            w2a = [ldw(f"w2a{l}") for l in range(LAYERS)]
            w2b = [ldw(f"w2b{l}") for l in range(LAYERS)]
            w3a = [ldw(f"w3a{l}") for l in range(LAYERS)]
            w3b = [ldw(f"w3b{l}") for l in range(LAYERS)]
            b1 = [ldb(f"b1{l}") for l in range(LAYERS)]
            b2 = [ldb(f"b2{l}") for l in range(LAYERS)]
            wih = [ldw(f"wih{l}") for l in range(LAYERS)]
            whh = [ldw(f"whh{l}") for l in range(LAYERS)]
            brz = [ldb(f"brz{l}") for l in range(LAYERS)]
            bgin = [ldb(f"bgin{l}") for l in range(LAYERS)]
            bghn = [ldb(f"bghn{l}") for l in range(LAYERS)]
            wd1 = ldw("wd1")
            wd2a = ldw("wd2a")
            wd2b = ldw("wd2b")
            wd3a = ldw("wd3a")
            wd3b = ldw("wd3b")
            bd1 = ldb("bd1")
            bd2 = ldb("bd2")

            iota = cpool.tile([128, 128], dt.float32, tag="iota")
            nc.gpsimd.iota(iota[:], pattern=[[1, 128]], base=0,
                           channel_multiplier=0,
                           allow_small_or_imprecise_dtypes=True)
            ident = cpool.tile([128, 128], dt.bfloat16, tag="ident")
            make_identity(nc, ident[:])

            h_f32 = spool.tile([128, SHP], dt.float32, tag="h_f32")
            h_bf = spool.tile([128, SHP], dt.bfloat16, tag="h_bf")
            h_nm = spool.tile([128, SHP], dt.bfloat16, tag="h_nm")
            agg_bf = spool.tile([128, SHP], dt.bfloat16, tag="agg_bf")

            def finish_layer():
                for b in range(NB):
                    tp = ppB.tile([128, 128], dt.bfloat16, tag="m3p")
                    nc.tensor.transpose(tp[:], h_bf[:, b * 128:(b + 1) * 128],
                                        ident[:])
                    nc.vector.tensor_copy(h_nm[:, b * 128:(b + 1) * 128], tp[:])
                nc.sync.dma_start(
                    out=loc_tab.ap().rearrange("(b p) d -> p b d", p=128),
                    in_=h_nm[:].rearrange("p (b d) -> p b d", d=D))
                nbf = SH // 128  # full 128-node blocks in the shard
                nc.sync.dma_start(
                    out=cc_in.ap()[:nbf * 128].rearrange("(b p) d -> p b d", p=128),
                    in_=h_nm[:, :nbf * 128].rearrange("p (b d) -> p b d", d=D))
                if SH > nbf * 128:
                    nc.sync.dma_start(
                        out=cc_in.ap()[nbf * 128:SH],
                        in_=h_nm[:SH - nbf * 128, nbf * 128:(nbf + 1) * 128])
                nc.gpsimd.collective_compute(
                    "AllGather", OP.bypass,
                    replica_groups=[list(range(NCORES))],
                    ins=[cc_in.ap()], outs=[table.ap()])

            # ---------------- encoder ----------------
            for n0, w in groups512(SHP):
                ps = ppA.tile([128, 512], dt.float32, tag="mp0")
                nc.tensor.matmul(ps[:, :w], lhsT=wenc[:], rhs=xsh[:, n0:n0 + w],
                                 start=True, stop=True)
                nc.scalar.activation(h_f32[:, n0:n0 + w], ps[:, :w], AF.Tanh,
                                     bias=benc[:, 0:1])
                nc.vector.tensor_copy(h_bf[:, n0:n0 + w], h_f32[:, n0:n0 + w])
            finish_layer()

            # ---------------- message-passing layers ----------------
            # repeat>1 re-runs the layer stack for timing (garbage numerics
            # after the first pass; used only by the benchmark).
            for l in [l for _ in range(repeat) for l in range(LAYERS)]:
                tile_idx = 0
                slot_off = 0
                for b in range(NB):
                    wl, wh = Bl[b] * 128, Bh[b] * 128
                    W = wl + wh
                    gsrc = gpool.tile([128, maxW], dt.bfloat16, tag="gsrc")
                    gtgt = gpool.tile([128, maxW], dt.bfloat16, tag="gtgt")
                    nc.gpsimd.dma_gather(
                        as3d(gsrc[:, 0:wl]), table.ap()[0:SPLIT],
                        idxrep[:, slot_off:slot_off + wl // 16],
                        wl, wl, D, transpose=True, single_packet=False)
                    nc.gpsimd.dma_gather(
                        as3d(gsrc[:, wl:W]), table.ap()[SPLIT:N],
                        idxrep[:, slot_off + wl // 16:slot_off + W // 16],
                        wh, wh, D, transpose=True, single_packet=False)
                    nc.gpsimd.dma_gather(
                        as3d(gtgt[:, 0:W]), loc_tab.ap(),
                        idxrep_tgt[:, slot_off:slot_off + W // 16],
                        W, W, D, transpose=True, single_packet=False)
                    slot_off += W // 16

                    aggp = ppC.tile([128, 128], dt.float32, tag="aggp")
                    first_tile = 0
                    for g0 in range(0, W, 1024):
                        w = min(1024, W - g0)
                        nt = w // 128
                        halves = [(h0, min(512, w - h0))
                                  for h0 in range(0, w, 512)]
                        m1s, m2s = [], []
                        for ci in range(2):
                            cs = slice(ci * 128, (ci + 1) * 128)
                            p = ppA.tile([128, 1024], dt.float32, tag=f"mp{ci}")
                            for h0, hw in halves:
                                nc.tensor.matmul(
                                    p[:, h0:h0 + hw], lhsT=w1t[l][:, cs],
                                    rhs=gtgt[:, g0 + h0:g0 + h0 + hw],
                                    start=True, stop=False)
                            for h0, hw in halves:
                                nc.tensor.matmul(
                                    p[:, h0:h0 + hw], lhsT=w1s[l][:, cs],
                                    rhs=gsrc[:, g0 + h0:g0 + h0 + hw],
                                    start=False, stop=True)
                            s = mpool.tile([128, 1024], dt.bfloat16,
                                           tag=f"m1s{ci}")
                            nc.scalar.activation(s[:, :w], p[:, :w], AF.Tanh,
                                                 bias=b1[l][:, ci:ci + 1])
                            m1s.append(s)
                        for ci in range(2):
                            cs = slice(ci * 128, (ci + 1) * 128)
                            p = ppA.tile([128, 1024], dt.float32, tag=f"mp{ci}")
                            for h0, hw in halves:
                                nc.tensor.matmul(
                                    p[:, h0:h0 + hw], lhsT=w2a[l][:, cs],
                                    rhs=m1s[0][:, h0:h0 + hw],
                                    start=True, stop=False)
                            for h0, hw in halves:
                                nc.tensor.matmul(
                                    p[:, h0:h0 + hw], lhsT=w2b[l][:, cs],
                                    rhs=m1s[1][:, h0:h0 + hw],
                                    start=False, stop=True)
                            s = mpool.tile([128, 1024], dt.bfloat16,
                                           tag=f"m2s{ci}")
                            nc.scalar.activation(s[:, :w], p[:, :w], AF.Tanh,
                                                 bias=b2[l][:, ci:ci + 1])
                            m2s.append(s)
                        m3p = ppB.tile([128, 1024], dt.float32, tag="m3p")
                        for t in range(nt):
                            ts = slice(t * 128, (t + 1) * 128)
                            nc.tensor.matmul(m3p[:, ts], lhsT=m2s[0][:, ts],
                                             rhs=w3a[l][:], start=True, stop=False)
                            nc.tensor.matmul(m3p[:, ts], lhsT=m2s[1][:, ts],
                                             rhs=w3b[l][:], start=False, stop=True)
                        m3s = mpool.tile([128, 1024], dt.bfloat16, tag="m3s")
                        nc.vector.tensor_copy(m3s[:, :w], m3p[:, :w])
                        psel = mpool.tile([128, 1024], dt.bfloat16, tag="psel")
                        for t in range(nt):
                            col = tile_idx + first_tile + t
                            nc.vector.tensor_scalar(
                                psel[:, t * 128:(t + 1) * 128], iota[:],
                                tgt_rel[:, col:col + 1], cinv[:, col:col + 1],
                                OP.is_equal, OP.mult)
                        for t in range(nt):
                            ts = slice(t * 128, (t + 1) * 128)
                            nc.tensor.matmul(
                                aggp[:], lhsT=m3s[:, ts], rhs=psel[:, ts],
                                start=(first_tile + t == 0),
                                stop=(first_tile + t == W // 128 - 1))
                        first_tile += nt
                    tile_idx += W // 128
                    nc.vector.tensor_copy(agg_bf[:, b * 128:(b + 1) * 128],
                                          aggp[:])

                # ---- GRU update over the node shard ----
                for n0, w in groups512(SHP):
                    ns = slice(n0, n0 + w)
                    rz = []
                    for k in range(2):
                        ks = slice(k * 128, (k + 1) * 128)
                        p = ppA.tile([128, 512], dt.float32, tag=f"mp{k}")
                        nc.tensor.matmul(p[:, :w], lhsT=wih[l][:, ks],
                                         rhs=agg_bf[:, ns], start=True, stop=False)
                        nc.tensor.matmul(p[:, :w], lhsT=whh[l][:, ks],
                                         rhs=h_bf[:, ns], start=False, stop=True)
                        s = mpool.tile([128, 512], dt.bfloat16, tag=f"m1s{k}")
                        nc.scalar.activation(s[:, :w], p[:, :w], AF.Sigmoid,
                                             bias=brz[l][:, k:k + 1])
                        rz.append(s)
                    gin = ppB.tile([128, 512], dt.float32, tag="m3p")
                    nc.tensor.matmul(gin[:, :w], lhsT=wih[l][:, 2 * 128:],
                                     rhs=agg_bf[:, ns], start=True, stop=True)
                    ghn = ppA.tile([128, 512], dt.float32, tag="mp0")
                    nc.tensor.matmul(ghn[:, :w], lhsT=whh[l][:, 2 * 128:],
                                     rhs=h_bf[:, ns], start=True, stop=True)
                    ghnb = mpool.tile([128, 512], dt.float32, tag="m2s0")
                    nc.vector.tensor_scalar(ghnb[:, :w], ghn[:, :w],
                                            bghn[l][:, 0:1], None, OP.add)
                    t1 = mpool.tile([128, 512], dt.float32, tag="m2s1")
                    nc.vector.tensor_tensor(t1[:, :w], rz[0][:, :w], ghnb[:, :w],
                                            OP.mult)
                    pre = mpool.tile([128, 512], dt.float32, tag="m3s")
                    nc.vector.tensor_tensor(pre[:, :w], gin[:, :w], t1[:, :w],
                                            OP.add)
                    nn = mpool.tile([128, 512], dt.float32, tag="psel")
                    nc.scalar.activation(nn[:, :w], pre[:, :w], AF.Tanh,
                                         bias=bgin[l][:, 0:1])
                    dd = mpool.tile([128, 512], dt.float32, tag="dd")
                    nc.vector.tensor_tensor(dd[:, :w], h_f32[:, ns], nn[:, :w],
                                            OP.subtract)
                    ee = mpool.tile([128, 512], dt.float32, tag="ee")
                    nc.vector.tensor_tensor(ee[:, :w], rz[1][:, :w], dd[:, :w],
                                            OP.mult)
                    nc.vector.tensor_tensor(h_f32[:, ns], nn[:, :w], ee[:, :w],
                                            OP.add)
                    nc.vector.tensor_copy(h_bf[:, ns], h_f32[:, ns])

                if l < LAYERS - 1:
                    finish_layer()

            # ---------------- decoder ----------------
            for n0, w in groups512(SHP):
                ns = slice(n0, n0 + w)
                o1s, o2s = [], []
                for ci in range(2):
                    cs = slice(ci * 128, (ci + 1) * 128)
                    p = ppA.tile([128, 512], dt.float32, tag=f"mp{ci}")
                    nc.tensor.matmul(p[:, :w], lhsT=wd1[:, cs],
                                     rhs=h_bf[:, ns], start=True, stop=True)
                    s = mpool.tile([128, 512], dt.bfloat16, tag=f"m1s{ci}")
                    nc.scalar.activation(s[:, :w], p[:, :w], AF.Tanh,
                                         bias=bd1[:, ci:ci + 1])
                    o1s.append(s)
                for ci in range(2):
                    cs = slice(ci * 128, (ci + 1) * 128)
                    p = ppA.tile([128, 512], dt.float32, tag=f"mp{ci}")
                    nc.tensor.matmul(p[:, :w], lhsT=wd2a[:, cs],
                                     rhs=o1s[0][:, :w], start=True, stop=False)
                    nc.tensor.matmul(p[:, :w], lhsT=wd2b[:, cs],
                                     rhs=o1s[1][:, :w], start=False, stop=True)
                    s = mpool.tile([128, 512], dt.bfloat16, tag=f"m2s{ci}")
                    nc.scalar.activation(s[:, :w], p[:, :w], AF.Tanh,
                                         bias=bd2[:, ci:ci + 1])
                    o2s.append(s)
                o3p = ppB.tile([1, 512], dt.float32, tag="m3p")
                nc.tensor.matmul(o3p[:, :w], lhsT=wd3a[:], rhs=o2s[0][:, :w],
                                 start=True, stop=False)
                nc.tensor.matmul(o3p[:, :w], lhsT=wd3b[:], rhs=o2s[1][:, :w],
                                 start=False, stop=True)
                yt = mpool.tile([1, 512], dt.float32, tag="m3s")
                nc.scalar.copy(yt[:, :w], o3p[:, :w])
                we = min(w, SH - n0) if n0 < SH else 0
                if we > 0:
                    nc.sync.dma_start(out=y_d.ap()[:, n0:n0 + we],
                                      in_=yt[:, :we])

    nc.compile()
    return nc


# ----------------------------------------------------------------------------
# Cached PJRT runner (same execute path run_bass_kernel_spmd takes under
# axon, but the jax.jit(shard_map(...)) callable is built once per program
# instead of per call, so repeat calls pay only staging + execution).
# ----------------------------------------------------------------------------

class _CachedRunner:
    def __init__(self, nc, n_cores):
        import jax
        from jax.sharding import Mesh, PartitionSpec
        from jax.experimental.shard_map import shard_map
        import concourse.mybir as mybir
        from concourse import bass2jax

        bass2jax.install_neuronx_cc_hook()
        self.n_cores = n_cores
        partition_name = (
            nc.partition_id_tensor.name if nc.partition_id_tensor else None)
        in_names, out_names, out_avals, zero_outs = [], [], [], []
        for alloc in nc.m.functions[0].allocations:
            if not isinstance(alloc, mybir.MemoryLocationSet):
                continue
            name = alloc.memorylocations[0].name
            if alloc.kind == "ExternalInput":
                if name != partition_name:
                    in_names.append(name)
            elif alloc.kind == "ExternalOutput":
                shape = tuple(alloc.tensor_shape)
                dtype = mybir.dt.np(alloc.dtype)
                out_names.append(name)
                out_avals.append(jax.core.ShapedArray(shape, dtype))
                zero_outs.append(np.zeros(shape, dtype))
        self.in_names = in_names
        self.out_names = out_names
        self.out_avals = out_avals
        self.zero_outs = zero_outs
        all_in = in_names + out_names
        if partition_name is not None:
            all_in.append(partition_name)

        def _body(*args):
            operands = list(args)
            if partition_name is not None:
                operands.append(bass2jax.partition_id_tensor())
            return tuple(bass2jax._bass_exec_p.bind(
                *operands,
                out_avals=tuple(out_avals),
                in_names=tuple(all_in),
                out_names=tuple(out_names),
                lowering_input_output_aliases=(),
                sim_require_finite=True,
                sim_require_nnan=True,
                nc=nc,
            ))

        donate = tuple(range(len(in_names),
                             len(in_names) + len(out_names)))
        devices = jax.devices()[:n_cores]
        mesh = Mesh(np.asarray(devices), ("core",))
        in_specs = (PartitionSpec("core"),) * (len(in_names) + len(out_names))
        out_specs = (PartitionSpec("core"),) * len(out_names)
        self._fn = jax.jit(
            shard_map(_body, mesh=mesh, in_specs=in_specs,
                      out_specs=out_specs, check_rep=False),
            donate_argnums=donate, keep_unused=True)

    def __call__(self, in_maps):
        n = self.n_cores
        concat_in = [
            np.concatenate([np.asarray(in_maps[c][name]) for c in range(n)],
                           axis=0)
            for name in self.in_names
        ]
        concat_zeros = [
            np.zeros((n * z.shape[0], *z.shape[1:]), z.dtype)
            for z in self.zero_outs
        ]
        out_arrs = self._fn(*concat_in, *concat_zeros)
        return [
            {name: np.asarray(out_arrs[i]).reshape(
                n, *self.out_avals[i].shape)[c]
             for i, name in enumerate(self.out_names)}
            for c in range(n)
        ]


# ----------------------------------------------------------------------------
# Entry point
# ----------------------------------------------------------------------------

def _prepare(inputs):
    meta, per_core = _prep(np.asarray(inputs["x"], np.float32),
                           np.asarray(inputs["edge_index"]))
    blob16, blob32 = _prep_weights(inputs)

    key = (tuple(meta["Bl"]), tuple(meta["Bh"]))
    if key not in _PROGRAM_CACHE:
        nc = _build_program(meta)
        _PROGRAM_CACHE[key] = (nc, _CachedRunner(nc, NCORES))
    nc, runner = _PROGRAM_CACHE[key]

    s16, s32 = TOT16 // NCORES, TOT32 // NCORES
    in_maps = []
    for c in range(NCORES):
        m = dict(per_core[c])
        m["wsh16"] = blob16[c * s16:(c + 1) * s16].reshape(1, s16)
        m["wsh32"] = blob32[c * s32:(c + 1) * s32].reshape(1, s32)
        in_maps.append(m)
    return runner, in_maps


def kernel(**inputs) -> np.ndarray:
    runner, in_maps = _prepare(inputs)
    res = runner(in_maps)
    out = np.concatenate([res[c]["y"][0] for c in range(NCORES)])
    return (out + np.asarray(inputs["dec_b3"], np.float32)[0]).astype(np.float32)


# revision 19
# speedup vs baseline: 36.8331x; 1.0226x over previous
"""Trainium2 Bass kernel for nn_MessagePassingGNN (8-core SPMD).

Strategy:
  - Sort edges (with self-loops) by target node; shard TARGET NODES across
    the 8 cores (6250 each) so each core owns a contiguous edge range and
    the segment-sum aggregation is core-local (no all-reduce).
  - Per layer, each core gathers source-node features from a replicated
    bf16 feature table in DRAM via dma_gather(transpose=True), which yields
    feature-major tiles that feed the message-MLP matmuls directly (no
    on-chip transposes). Target-side gathers read a core-local shard table
    so they never wait on the collective.
  - The scatter-mean aggregation runs on the tensor engine: a scaled one-hot
    matrix P[e, n] = (tgt_rel[e] == n) / count[tgt_e] is built by one fused
    DVE tensor_scalar per 128-edge tile, then agg += m3_tile.T @ P_tile
    accumulates in PSUM per 128-target-node block.
  - GRU update is node-sharded; updated shard features are AllGather'd into
    every core's table for the next layer. The decoder runs on the local
    shard; the host concatenates the 8 shards.

Host<->device transfer is minimized (it dominates wall-clock through the
axon tunnel):
  - dma_gather index tables are staged compactly as [16, W/16] and
    replicated across the 8 partition groups on-device (8x fewer bytes).
  - Per-edge target-relative ids and segment counts ship as int8 and are
    converted / reciprocated on-device.
  - All weights/biases ship as two flat blobs, each sharded 1/8th per core,
    and are reassembled on-device with an AllGather collective (8x fewer
    bytes than replicating them).
  - The PJRT executable for the Bass program is built once and cached, so
    repeat calls pay only input staging + device execution (this matches
    what run_bass_kernel_spmd does under axon, minus the per-call
    jax.jit/shard_map rebuild).

All matmuls are bf16 with fp32 PSUM accumulation; GRU elementwise math is
fp32. Host-measured end-to-end L2 relative error vs fp32 reference ~1e-2.
"""

import math

import numpy as np
import ml_dtypes

# Problem constants (hardcoded per harness contract).
N, IN_DIM, D, H, E, LAYERS = 50000, 16, 128, 256, 800000, 3
NCORES = 8
SH = N // NCORES            # 6250 nodes per shard
NB = (SH + 127) // 128      # 49 blocks of 128 target nodes
SHP = NB * 128              # 6272 padded shard width
SPLIT = 32768               # int16 index split for the gather table
BF16 = ml_dtypes.bfloat16

_PROGRAM_CACHE = {}


# ----------------------------------------------------------------------------
# Weight blob layout (static; shared by host packer and device program)
# ----------------------------------------------------------------------------

def _blob_layout():
    L16 = [("wenc", (IN_DIM, 128))]
    for l in range(LAYERS):
        L16 += [(f"w1t{l}", (128, H)), (f"w1s{l}", (128, H)),
                (f"w2a{l}", (128, H)), (f"w2b{l}", (128, H)),
                (f"w3a{l}", (128, D)), (f"w3b{l}", (128, D)),
                (f"wih{l}", (128, 3 * D)), (f"whh{l}", (128, 3 * D))]
    L16 += [("wd1", (128, H)), ("wd2a", (128, H)), ("wd2b", (128, H)),
            ("wd3a", (128, 1)), ("wd3b", (128, 1))]
    L32 = [("benc", (128, 1))]
    for l in range(LAYERS):
        L32 += [(f"b1{l}", (128, 2)), (f"b2{l}", (128, 2)),
                (f"brz{l}", (128, 2)), (f"bgin{l}", (128, 1)),
                (f"bghn{l}", (128, 1))]
    L32 += [("bd1", (128, 2)), ("bd2", (128, 2))]

    def offsets(items):
        offs, o = {}, 0
        for name, shp in items:
            offs[name] = (o, shp)
            o += shp[0] * shp[1]
        return offs, o + ((-o) % (NCORES * 128))

    O16, T16 = offsets(L16)
    O32, T32 = offsets(L32)
    return O16, T16, O32, T32


OFF16, TOT16, OFF32, TOT32 = _blob_layout()


# ----------------------------------------------------------------------------
# Host-side preprocessing
# ----------------------------------------------------------------------------

def _wrap16(idx_i16):
    """dma_gather index layout: index i lives at [i % 16, i // 16]. The
    8x partition-group replication happens on-device."""
    n = idx_i16.shape[0]
    return idx_i16.reshape(n // 16, 16).T


def _prep(x, edge_index):
    loops = np.arange(N, dtype=np.int64)
    src = np.concatenate([np.asarray(edge_index[0]), loops])
    tgt = np.concatenate([np.asarray(edge_index[1]), loops])
    order = np.argsort(tgt, kind="stable")
    src_s = src[order].astype(np.int32)
    tgt_s = tgt[order].astype(np.int32)
    counts = np.zeros(N, np.int32)
    np.add.at(counts, tgt_s, 1)
    assert counts.max() < 128, "int8 staging assumes max degree < 128"

    node_starts = np.searchsorted(tgt_s, np.arange(N + 1))
    lows = np.zeros((NCORES, NB), np.int64)
    highs = np.zeros((NCORES, NB), np.int64)
    rng = {}
    for c in range(NCORES):
        for b in range(NB):
            lo_node = c * SH + b * 128
            hi_node = min(c * SH + SH, lo_node + 128)
            e0, e1 = node_starts[lo_node], node_starts[hi_node]
            nl = int((src_s[e0:e1] < SPLIT).sum())
            lows[c, b] = nl
            highs[c, b] = (e1 - e0) - nl
            rng[(c, b)] = (e0, e1)
    Bl = [int(max(1, math.ceil(lows[:, b].max() / 128))) for b in range(NB)]
    Bh = [int(max(1, math.ceil(highs[:, b].max() / 128))) for b in range(NB)]

    meta = {"Bl": Bl, "Bh": Bh}
    ntiles = sum(Bl) + sum(Bh)
    nslots = ntiles * 128

    per_core = []
    for c in range(NCORES):
        idx_src = np.zeros(nslots, np.int16)
        tgt_rel = np.full(nslots, -1, np.int8)
        cnt_e = np.ones(nslots, np.int8)
        off = 0
        for b in range(NB):
            e0, e1 = rng[(c, b)]
            s, t = src_s[e0:e1], tgt_s[e0:e1]
            lo = s < SPLIT
            for mask, cap, base in ((lo, Bl[b], 0), (~lo, Bh[b], SPLIT)):
                sh_, th_ = s[mask], t[mask]
                n = sh_.shape[0]
                idx_src[off:off + n] = (sh_ - base).astype(np.int16)
                tgt_rel[off:off + n] = (th_ - (c * SH + b * 128)).astype(np.int8)
                cnt_e[off:off + n] = counts[th_].astype(np.int8)
                off += cap * 128
        assert off == nslots

        src_cols = []
        off = 0
        for b in range(NB):
            wl, wh = Bl[b] * 128, Bh[b] * 128
            src_cols.append(_wrap16(idx_src[off:off + wl]))
            src_cols.append(_wrap16(idx_src[off + wl:off + wl + wh]))
            off += wl + wh

        xs = np.zeros((IN_DIM, SHP), np.float32)
        xs[:, :SH] = np.asarray(x[c * SH:(c + 1) * SH]).T
        per_core.append({
            "x_sh_t": xs.astype(BF16),
            "idx16": np.concatenate(src_cols, axis=1),
            "aux8": np.concatenate(
                [_wrap16(tgt_rel), _wrap16(cnt_e)], axis=1).copy(),
        })
    return meta, per_core


def _prep_weights(inp):
    f32 = np.float32
    bf = lambda a: np.ascontiguousarray(np.asarray(a, f32)).astype(BF16)
    w = {}
    w["wenc"] = bf(inp["enc_W"])
    w["benc"] = np.asarray(inp["enc_b"], f32).reshape(128, 1)
    for l in range(LAYERS):
        w[f"w1t{l}"] = bf(inp["msg_W1"][l][:D, :])
        w[f"w1s{l}"] = bf(inp["msg_W1"][l][D:, :])
        w[f"w2a{l}"] = bf(inp["msg_W2"][l][:128, :])
        w[f"w2b{l}"] = bf(inp["msg_W2"][l][128:, :])
        w[f"w3a{l}"] = bf(inp["msg_W3"][l][:128, :])
        w[f"w3b{l}"] = bf(inp["msg_W3"][l][128:, :])
        w[f"b1{l}"] = np.asarray(inp["msg_b1"][l], f32).reshape(2, 128).T
        w[f"b2{l}"] = np.asarray(inp["msg_b2"][l], f32).reshape(2, 128).T
        w[f"wih{l}"] = bf(inp["gru_Wih"][l])
        w[f"whh{l}"] = bf(inp["gru_Whh"][l])
        bgi = (np.asarray(inp["msg_b3"][l], f32)
               @ np.asarray(inp["gru_Wih"][l], f32)
               + np.asarray(inp["gru_bih"][l], f32))
        bhh = np.asarray(inp["gru_bhh"][l], f32)
        w[f"brz{l}"] = (bgi[:2 * D] + bhh[:2 * D]).reshape(2, 128).T
        w[f"bgin{l}"] = bgi[2 * D:].reshape(128, 1)
        w[f"bghn{l}"] = bhh[2 * D:].reshape(128, 1)
    w["wd1"] = bf(inp["dec_W1"])
    w["wd2a"] = bf(inp["dec_W2"][:128, :])
    w["wd2b"] = bf(inp["dec_W2"][128:, :])
    w["wd3a"] = bf(inp["dec_W3"][:128, :])
    w["wd3b"] = bf(inp["dec_W3"][128:, :])
    w["bd1"] = np.asarray(inp["dec_b1"], f32).reshape(2, 128).T
    w["bd2"] = np.asarray(inp["dec_b2"], f32).reshape(2, 128).T

    blob16 = np.zeros(TOT16, BF16)
    for name, (off, shp) in OFF16.items():
        blob16[off:off + shp[0] * shp[1]] = w[name].reshape(-1)
    blob32 = np.zeros(TOT32, f32)
    for name, (off, shp) in OFF32.items():
        blob32[off:off + shp[0] * shp[1]] = w[name].reshape(-1)
    return blob16, blob32


# ----------------------------------------------------------------------------
# Bass program
# ----------------------------------------------------------------------------

def _build_program(meta, debug=False, repeat=1):
    import concourse.bacc as bacc
    import concourse.mybir as mybir
    import concourse.tile as tile
    from concourse import library_config
    from concourse.masks import make_identity
    from concourse.tile_rust import add_dep_helper

    Bl, Bh = meta["Bl"], meta["Bh"]
    ntiles = sum(Bl) + sum(Bh)
    nslots = ntiles * 128
    T = nslots // 16            # wrapped-layout column count
    maxW = max((Bl[b] + Bh[b]) * 128 for b in range(NB))
    dt = mybir.dt
    AF = mybir.ActivationFunctionType
    OP = mybir.AluOpType

    nc = bacc.Bacc("TRN2", target_bir_lowering=False, debug=debug,
                   num_devices=NCORES)

    ext_in = lambda n, s, d: nc.dram_tensor(n, s, d, kind="ExternalInput")
    x_sh_t = ext_in("x_sh_t", [IN_DIM, SHP], dt.bfloat16)
    idx16_d = ext_in("idx16", [16, T], dt.int16)
    aux8_d = ext_in("aux8", [16, 2 * T], dt.int8)
    wsh16_d = ext_in("wsh16", [1, TOT16 // NCORES], dt.bfloat16)
    wsh32_d = ext_in("wsh32", [1, TOT32 // NCORES], dt.float32)
    y_d = nc.dram_tensor("y", [1, SH], dt.float32, kind="ExternalOutput")

    wtmp16 = nc.dram_tensor("wtmp16", [1, TOT16 // NCORES], dt.bfloat16)
    wtmp32 = nc.dram_tensor("wtmp32", [1, TOT32 // NCORES], dt.float32)
    wfull16 = nc.dram_tensor("wfull16", [1, TOT16], dt.bfloat16,
                             addr_space="Shared")
    wfull32 = nc.dram_tensor("wfull32", [1, TOT32], dt.float32,
                             addr_space="Shared")
    table = nc.dram_tensor("table", [N, D], dt.bfloat16, addr_space="Shared")
    cc_in = nc.dram_tensor("cc_in", [SH, D], dt.bfloat16)

    groups512 = lambda W: [(g0, min(512, W - g0)) for g0 in range(0, W, 512)]
    as3d = lambda ap: ap.rearrange("p (o n) -> p o n", o=1)

    with tile.TileContext(nc, num_cores=NCORES) as tc:
        nc.gpsimd.load_library(library_config.mlp)

        with (
            tc.tile_pool(name="const", bufs=1) as cpool,
            tc.tile_pool(name="state", bufs=1) as spool,
            tc.tile_pool(name="gather", bufs=2) as gpool,
            tc.tile_pool(name="mlp", bufs=2) as mpool,
            tc.tile_pool(name="psA", bufs=1, space="PSUM") as ppA,
            tc.tile_pool(name="psB", bufs=1, space="PSUM") as ppB,
            tc.tile_pool(name="psC", bufs=2, space="PSUM") as ppC,
        ):
            # -------- reassemble the replicated weight blobs on-device ----
            # (collectives can't read IO tensors; bounce through internal DRAM)
            ld16 = nc.sync.dma_start(out=wtmp16.ap(), in_=wsh16_d.ap())
            ld32 = nc.sync.dma_start(out=wtmp32.ap(), in_=wsh32_d.ap())
            cc16 = nc.gpsimd.collective_compute(
                "AllGather", OP.bypass,
                replica_groups=[list(range(NCORES))],
                ins=[wtmp16.ap()], outs=[wfull16.ap()])
            cc32 = nc.gpsimd.collective_compute(
                "AllGather", OP.bypass,
                replica_groups=[list(range(NCORES))],
                ins=[wtmp32.ap()], outs=[wfull32.ap()])
            # DRAM RAW hazards aren't tracked by tile's shadow memory
            # (SBUF/PSUM only) — declare the edges explicitly.
            add_dep_helper(cc16.ins, ld16.ins, reason="allgather after stage")
            add_dep_helper(cc32.ins, ld32.ins, reason="allgather after stage")

            def ldw(nm):
                off, (p, w) = OFF16[nm]
                t = cpool.tile([p, w], dt.bfloat16, tag=nm)
                di = nc.sync.dma_start(
                    out=t[:],
                    in_=wfull16.ap()[0, off:off + p * w]
                        .rearrange("(p w) -> p w", w=w))
                add_dep_helper(di.ins, cc16.ins, reason="load after allgather")
                return t

            def ldb(nm):
                off, (p, w) = OFF32[nm]
                t = cpool.tile([p, w], dt.float32, tag=nm)
                di = nc.sync.dma_start(
                    out=t[:],
                    in_=wfull32.ap()[0, off:off + p * w]
                        .rearrange("(p w) -> p w", w=w))
                add_dep_helper(di.ins, cc32.ins, reason="load after allgather")
                return t

            def ld(dram_ap, nm, dtype=None):
                t = cpool.tile(list(dram_ap.shape), dtype or dram_ap.dtype,
                               tag=nm)
                nc.sync.dma_start(out=t[:], in_=dram_ap)
                return t

            xsh = ld(x_sh_t.ap(), "xsh")

            # replicate the compact [16, T] gather-index / tgt-rel tables
            # across the 8 partition groups (dma_gather wants them in all)
            idxrep = cpool.tile([128, T], dt.int16, tag="idxrep")
            trw_rep = cpool.tile([128, T], dt.int8, tag="trw_rep")
            for k in range(8):
                nc.sync.dma_start(out=idxrep[16 * k:16 * (k + 1), :],
                                  in_=idx16_d.ap())
                nc.sync.dma_start(out=trw_rep[16 * k:16 * (k + 1), :],
                                  in_=aux8_d.ap()[:, 0:T])

            # int16 target-gather indices: clamp(tgt_rel, 0, 127) + 128*block.
            # The clamp sends the pad slots (-1, or 255 if the int8 widen is
            # unsigned) to a real in-range node; the psel mask below still
            # sees the raw pad value and zeroes their contribution.
            idxrep_tgt = cpool.tile([128, T], dt.int16, tag="idxrep_tgt")
            nc.vector.tensor_copy(idxrep_tgt[:], trw_rep[:])
            nc.vector.tensor_scalar(idxrep_tgt[:], idxrep_tgt[:], 0, 127,
                                    OP.max, OP.min)
            c0 = 0
            for b in range(NB):
                c1 = c0 + (Bl[b] + Bh[b]) * 8
                if b:
                    nc.vector.tensor_scalar(idxrep_tgt[:, c0:c1],
                                            idxrep_tgt[:, c0:c1],
                                            128 * b, None, OP.add)
                c0 = c1

            # per-edge-slot tgt_rel / count in partitioned [e%128, e//128]
            # layout: wrapped (p, 8c+k) -> partitioned (16k+p, c)
            tr8p = cpool.tile([128, ntiles], dt.int8, tag="tr8p")
            cnt8p = cpool.tile([128, ntiles], dt.int8, tag="cnt8p")
            for k in range(8):
                nc.sync.dma_start(out=tr8p[16 * k:16 * (k + 1), :],
                                  in_=aux8_d.ap()[:, k:T:8])
                nc.sync.dma_start(out=cnt8p[16 * k:16 * (k + 1), :],
                                  in_=aux8_d.ap()[:, T + k:2 * T:8])
            tgt_rel = cpool.tile([128, ntiles], dt.float32, tag="tgt_rel")
            nc.vector.tensor_copy(tgt_rel[:], tr8p[:])
            cntf = cpool.tile([128, ntiles], dt.float32, tag="cntf")
            nc.vector.tensor_copy(cntf[:], cnt8p[:])
            cinv = cpool.tile([128, ntiles], dt.float32, tag="cinv")
            nc.vector.reciprocal(cinv[:], cntf[:])

            wenc = ldw("wenc")
            benc = ldb("benc")
            w1t = [ldw(f"w1t{l}") for l in range(LAYERS)]
            w1s = [ldw(f"w1s{l}") for l in range(LAYERS)]
            w2a = [ldw(f"w2a{l}") for l in range(LAYERS)]
            w2b = [ldw(f"w2b{l}") for l in range(LAYERS)]
            w3a = [ldw(f"w3a{l}") for l in range(LAYERS)]
            w3b = [ldw(f"w3b{l}") for l in range(LAYERS)]
            b1 = [ldb(f"b1{l}") for l in range(LAYERS)]
            b2 = [ldb(f"b2{l}") for l in range(LAYERS)]
            wih = [ldw(f"wih{l}") for l in range(LAYERS)]
            whh = [ldw(f"whh{l}") for l in range(LAYERS)]
            brz = [ldb(f"brz{l}") for l in range(LAYERS)]
            bgin = [ldb(f"bgin{l}") for l in range(LAYERS)]
            bghn = [ldb(f"bghn{l}") for l in range(LAYERS)]
            wd1 = ldw("wd1")
            wd2a = ldw("wd2a")
            wd2b = ldw("wd2b")
            wd3a = ldw("wd3a")
            wd3b = ldw("wd3b")
            bd1 = ldb("bd1")
            bd2 = ldb("bd2")

            iota = cpool.tile([128, 128], dt.float32, tag="iota")
            nc.gpsimd.iota(iota[:], pattern=[[1, 128]], base=0,
                           channel_multiplier=0,
                           allow_small_or_imprecise_dtypes=True)
            ident = cpool.tile([128, 128], dt.bfloat16, tag="ident")
            make_identity(nc, ident[:])

            h_f32 = spool.tile([128, SHP], dt.float32, tag="h_f32")
            h_bf = spool.tile([128, SHP], dt.bfloat16, tag="h_bf")
            h_nm = spool.tile([128, SHP], dt.bfloat16, tag="h_nm")
            agg_bf = spool.tile([128, SHP], dt.bfloat16, tag="agg_bf")

            def finish_layer():
                for b in range(NB):
                    tp = ppB.tile([128, 128], dt.bfloat16, tag="m3p")
                    nc.tensor.transpose(tp[:], h_bf[:, b * 128:(b + 1) * 128],
                                        ident[:])
                    nc.vector.tensor_copy(h_nm[:, b * 128:(b + 1) * 128], tp[:])
                nbf = SH // 128  # full 128-node blocks in the shard
                nc.sync.dma_start(
                    out=cc_in.ap()[:nbf * 128].rearrange("(b p) d -> p b d", p=128),
                    in_=h_nm[:, :nbf * 128].rearrange("p (b d) -> p b d", d=D))
                if SH > nbf * 128:
                    nc.sync.dma_start(
                        out=cc_in.ap()[nbf * 128:SH],
                        in_=h_nm[:SH - nbf * 128, nbf * 128:(nbf + 1) * 128])
                nc.gpsimd.collective_compute(
                    "AllGather", OP.bypass,
                    replica_groups=[list(range(NCORES))],
                    ins=[cc_in.ap()], outs=[table.ap()])

            # ---------------- encoder ----------------
            for n0, w in groups512(SHP):
                ps = ppA.tile([128, 512], dt.float32, tag="mp0")
                nc.tensor.matmul(ps[:, :w], lhsT=wenc[:], rhs=xsh[:, n0:n0 + w],
                                 start=True, stop=True)
                nc.scalar.activation(h_f32[:, n0:n0 + w], ps[:, :w], AF.Tanh,
                                     bias=benc[:, 0:1])
                nc.vector.tensor_copy(h_bf[:, n0:n0 + w], h_f32[:, n0:n0 + w])
            finish_layer()

            # ---------------- message-passing layers ----------------
            # repeat>1 re-runs the layer stack for timing (garbage numerics
            # after the first pass; used only by the benchmark).
            for l in [l for _ in range(repeat) for l in range(LAYERS)]:
                tile_idx = 0
                slot_off = 0
                for b in range(NB):
                    wl, wh = Bl[b] * 128, Bh[b] * 128
                    W = wl + wh
                    gsrc = gpool.tile([128, maxW], dt.bfloat16, tag="gsrc")
                    gtgt = gpool.tile([128, maxW], dt.bfloat16, tag="gtgt")
                    nc.gpsimd.dma_gather(
                        as3d(gsrc[:, 0:wl]), table.ap()[0:SPLIT],
                        idxrep[:, slot_off:slot_off + wl // 16],
                        wl, wl, D, transpose=True, single_packet=False)
                    nc.gpsimd.dma_gather(
                        as3d(gsrc[:, wl:W]), table.ap()[SPLIT:N],
                        idxrep[:, slot_off + wl // 16:slot_off + W // 16],
                        wh, wh, D, transpose=True, single_packet=False)
                    # target features gathered straight out of the node-major
                    # SBUF copy (node n: partition n%128, 256B stripe n//128)
                    nc.gpsimd.dma_gather(
                        as3d(gtgt[:, 0:W]), h_nm[:],
                        idxrep_tgt[:, slot_off:slot_off + W // 16],
                        W, W, D, transpose=True, single_packet=False,
                        sbuf_tokens_per_rank=128,
                        sbuf_free_dim_per_rank=256)
                    slot_off += W // 16

                    aggp = ppC.tile([128, 128], dt.float32, tag="aggp")
                    first_tile = 0
                    for g0 in range(0, W, 1024):
                        w = min(1024, W - g0)
                        nt = w // 128
                        halves = [(h0, min(512, w - h0))
                                  for h0 in range(0, w, 512)]
                        m1s, m2s = [], []
                        for ci in range(2):
                            cs = slice(ci * 128, (ci + 1) * 128)
                            p = ppA.tile([128, 1024], dt.float32, tag=f"mp{ci}")
                            for h0, hw in halves:
                                nc.tensor.matmul(
                                    p[:, h0:h0 + hw], lhsT=w1t[l][:, cs],
                                    rhs=gtgt[:, g0 + h0:g0 + h0 + hw],
                                    start=True, stop=False)
                            for h0, hw in halves:
                                nc.tensor.matmul(
                                    p[:, h0:h0 + hw], lhsT=w1s[l][:, cs],
                                    rhs=gsrc[:, g0 + h0:g0 + h0 + hw],
                                    start=False, stop=True)
                            s = mpool.tile([128, 1024], dt.bfloat16,
                                           tag=f"m1s{ci}")
                            nc.scalar.activation(s[:, :w], p[:, :w], AF.Tanh,
                                                 bias=b1[l][:, ci:ci + 1])
                            m1s.append(s)
                        for ci in range(2):
                            cs = slice(ci * 128, (ci + 1) * 128)
                            p = ppA.tile([128, 1024], dt.float32, tag=f"mp{ci}")
                            for h0, hw in halves:
                                nc.tensor.matmul(
                                    p[:, h0:h0 + hw], lhsT=w2a[l][:, cs],
                                    rhs=m1s[0][:, h0:h0 + hw],
                                    start=True, stop=False)
                            for h0, hw in halves:
                                nc.tensor.matmul(
                                    p[:, h0:h0 + hw], lhsT=w2b[l][:, cs],
                                    rhs=m1s[1][:, h0:h0 + hw],
                                    start=False, stop=True)
                            s = mpool.tile([128, 1024], dt.bfloat16,
                                           tag=f"m2s{ci}")
                            nc.scalar.activation(s[:, :w], p[:, :w], AF.Tanh,
                                                 bias=b2[l][:, ci:ci + 1])
                            m2s.append(s)
                        m3p = ppB.tile([128, 1024], dt.float32, tag="m3p")
                        for t in range(nt):
                            ts = slice(t * 128, (t + 1) * 128)
                            nc.tensor.matmul(m3p[:, ts], lhsT=m2s[0][:, ts],
                                             rhs=w3a[l][:], start=True, stop=False)
                            nc.tensor.matmul(m3p[:, ts], lhsT=m2s[1][:, ts],
                                             rhs=w3b[l][:], start=False, stop=True)
                        m3s = mpool.tile([128, 1024], dt.bfloat16, tag="m3s")
                        nc.vector.tensor_copy(m3s[:, :w], m3p[:, :w])
                        psel = mpool.tile([128, 1024], dt.bfloat16, tag="psel")
                        for t in range(nt):
                            col = tile_idx + first_tile + t
                            nc.vector.tensor_scalar(
                                psel[:, t * 128:(t + 1) * 128], iota[:],
                                tgt_rel[:, col:col + 1], cinv[:, col:col + 1],
                                OP.is_equal, OP.mult)
                        for t in range(nt):
                            ts = slice(t * 128, (t + 1) * 128)
                            nc.tensor.matmul(
                                aggp[:], lhsT=m3s[:, ts], rhs=psel[:, ts],
                                start=(first_tile + t == 0),
                                stop=(first_tile + t == W // 128 - 1))
                        first_tile += nt
                    tile_idx += W // 128
                    nc.vector.tensor_copy(agg_bf[:, b * 128:(b + 1) * 128],
                                          aggp[:])

                # ---- GRU update over the node shard ----
                for n0, w in groups512(SHP):
                    ns = slice(n0, n0 + w)
                    rz = []
                    for k in range(2):
                        ks = slice(k * 128, (k + 1) * 128)
                        p = ppA.tile([128, 512], dt.float32, tag=f"mp{k}")
                        nc.tensor.matmul(p[:, :w], lhsT=wih[l][:, ks],
                                         rhs=agg_bf[:, ns], start=True, stop=False)
                        nc.tensor.matmul(p[:, :w], lhsT=whh[l][:, ks],
                                         rhs=h_bf[:, ns], start=False, stop=True)
                        s = mpool.tile([128, 512], dt.bfloat16, tag=f"m1s{k}")
                        nc.scalar.activation(s[:, :w], p[:, :w], AF.Sigmoid,
                                             bias=brz[l][:, k:k + 1])
                        rz.append(s)
                    gin = ppB.tile([128, 512], dt.float32, tag="m3p")
                    nc.tensor.matmul(gin[:, :w], lhsT=wih[l][:, 2 * 128:],
                                     rhs=agg_bf[:, ns], start=True, stop=True)
                    ghn = ppA.tile([128, 512], dt.float32, tag="mp0")
                    nc.tensor.matmul(ghn[:, :w], lhsT=whh[l][:, 2 * 128:],
                                     rhs=h_bf[:, ns], start=True, stop=True)
                    ghnb = mpool.tile([128, 512], dt.float32, tag="m2s0")
                    nc.vector.tensor_scalar(ghnb[:, :w], ghn[:, :w],
                                            bghn[l][:, 0:1], None, OP.add)
                    t1 = mpool.tile([128, 512], dt.float32, tag="m2s1")
                    nc.vector.tensor_tensor(t1[:, :w], rz[0][:, :w], ghnb[:, :w],
                                            OP.mult)
                    pre = mpool.tile([128, 512], dt.float32, tag="m3s")
                    nc.vector.tensor_tensor(pre[:, :w], gin[:, :w], t1[:, :w],
                                            OP.add)
                    nn = mpool.tile([128, 512], dt.float32, tag="psel")
                    nc.scalar.activation(nn[:, :w], pre[:, :w], AF.Tanh,
                                         bias=bgin[l][:, 0:1])
                    dd = mpool.tile([128, 512], dt.float32, tag="dd")
                    nc.vector.tensor_tensor(dd[:, :w], h_f32[:, ns], nn[:, :w],
                                            OP.subtract)
                    ee = mpool.tile([128, 512], dt.float32, tag="ee")
                    nc.vector.tensor_tensor(ee[:, :w], rz[1][:, :w], dd[:, :w],
                                            OP.mult)
                    nc.vector.tensor_tensor(h_f32[:, ns], nn[:, :w], ee[:, :w],
                                            OP.add)
                    nc.vector.tensor_copy(h_bf[:, ns], h_f32[:, ns])

                if l < LAYERS - 1:
                    finish_layer()

            # ---------------- decoder ----------------
            for n0, w in groups512(SHP):
                ns = slice(n0, n0 + w)
                o1s, o2s = [], []
                for ci in range(2):
                    cs = slice(ci * 128, (ci + 1) * 128)
                    p = ppA.tile([128, 512], dt.float32, tag=f"mp{ci}")
                    nc.tensor.matmul(p[:, :w], lhsT=wd1[:, cs],
                                     rhs=h_bf[:, ns], start=True, stop=True)
                    s = mpool.tile([128, 512], dt.bfloat16, tag=f"m1s{ci}")
                    nc.scalar.activation(s[:, :w], p[:, :w], AF.Tanh,
                                         bias=bd1[:, ci:ci + 1])
                    o1s.append(s)
                for ci in range(2):
                    cs = slice(ci * 128, (ci + 1) * 128)
                    p = ppA.tile([128, 512], dt.float32, tag=f"mp{ci}")
                    nc.tensor.matmul(p[:, :w], lhsT=wd2a[:, cs],
                                     rhs=o1s[0][:, :w], start=True, stop=False)
                    nc.tensor.matmul(p[:, :w], lhsT=wd2b[:, cs],
                                     rhs=o1s[1][:, :w], start=False, stop=True)
                    s = mpool.tile([128, 512], dt.bfloat16, tag=f"m2s{ci}")
                    nc.scalar.activation(s[:, :w], p[:, :w], AF.Tanh,
                                         bias=bd2[:, ci:ci + 1])
                    o2s.append(s)
                o3p = ppB.tile([1, 512], dt.float32, tag="m3p")
                nc.tensor.matmul(o3p[:, :w], lhsT=wd3a[:], rhs=o2s[0][:, :w],
                                 start=True, stop=False)
                nc.tensor.matmul(o3p[:, :w], lhsT=wd3b[:], rhs=o2s[1][:, :w],
                                 start=False, stop=True)
                yt = mpool.tile([1, 512], dt.float32, tag="m3s")
                nc.scalar.copy(yt[:, :w], o3p[:, :w])
                we = min(w, SH - n0) if n0 < SH else 0
                if we > 0:
                    nc.sync.dma_start(out=y_d.ap()[:, n0:n0 + we],
                                      in_=yt[:, :we])

    nc.compile()
    return nc


# ----------------------------------------------------------------------------
# Cached PJRT runner (same execute path run_bass_kernel_spmd takes under
# axon, but the jax.jit(shard_map(...)) callable is built once per program
# instead of per call, so repeat calls pay only staging + execution).
# ----------------------------------------------------------------------------

class _CachedRunner:
    def __init__(self, nc, n_cores):
        import jax
        from jax.sharding import Mesh, PartitionSpec
        from jax.experimental.shard_map import shard_map
        import concourse.mybir as mybir
        from concourse import bass2jax

        bass2jax.install_neuronx_cc_hook()
        self.n_cores = n_cores
        partition_name = (
            nc.partition_id_tensor.name if nc.partition_id_tensor else None)
        in_names, out_names, out_avals, zero_outs = [], [], [], []
        for alloc in nc.m.functions[0].allocations:
            if not isinstance(alloc, mybir.MemoryLocationSet):
                continue
            name = alloc.memorylocations[0].name
            if alloc.kind == "ExternalInput":
                if name != partition_name:
                    in_names.append(name)
            elif alloc.kind == "ExternalOutput":
                shape = tuple(alloc.tensor_shape)
                dtype = mybir.dt.np(alloc.dtype)
                out_names.append(name)
                out_avals.append(jax.core.ShapedArray(shape, dtype))
                zero_outs.append(np.zeros(shape, dtype))
        self.in_names = in_names
        self.out_names = out_names
        self.out_avals = out_avals
        self.zero_outs = zero_outs
        all_in = in_names + out_names
        if partition_name is not None:
            all_in.append(partition_name)

        def _body(*args):
            operands = list(args)
            if partition_name is not None:
                operands.append(bass2jax.partition_id_tensor())
            return tuple(bass2jax._bass_exec_p.bind(
                *operands,
                out_avals=tuple(out_avals),
                in_names=tuple(all_in),
                out_names=tuple(out_names),
                lowering_input_output_aliases=(),
                sim_require_finite=True,
                sim_require_nnan=True,
                nc=nc,
            ))

        donate = tuple(range(len(in_names),
                             len(in_names) + len(out_names)))
        devices = jax.devices()[:n_cores]
        mesh = Mesh(np.asarray(devices), ("core",))
        in_specs = (PartitionSpec("core"),) * (len(in_names) + len(out_names))
        out_specs = (PartitionSpec("core"),) * len(out_names)
        self._fn = jax.jit(
            shard_map(_body, mesh=mesh, in_specs=in_specs,
                      out_specs=out_specs, check_rep=False),
            donate_argnums=donate, keep_unused=True)

    def stack(self, in_maps):
        n = self.n_cores
        return [
            np.concatenate([np.asarray(in_maps[c][name]) for c in range(n)],
                           axis=0)
            for name in self.in_names
        ]

    def run_stacked(self, stacked):
        n = self.n_cores
        concat_zeros = [
            np.zeros((n * z.shape[0], *z.shape[1:]), z.dtype)
            for z in self.zero_outs
        ]
        out_arrs = self._fn(*stacked, *concat_zeros)
        for a in out_arrs:
            a.copy_to_host_async()
        return [
            {name: np.asarray(out_arrs[i]).reshape(
                n, *self.out_avals[i].shape)[c]
             for i, name in enumerate(self.out_names)}
            for c in range(n)
        ]

    def __call__(self, in_maps):
        return self.run_stacked(self.stack(in_maps))


# ----------------------------------------------------------------------------
# Entry point
# ----------------------------------------------------------------------------

def _prepare(inputs):
    meta, per_core = _prep(np.asarray(inputs["x"], np.float32),
                           np.asarray(inputs["edge_index"]))
    blob16, blob32 = _prep_weights(inputs)

    key = (tuple(meta["Bl"]), tuple(meta["Bh"]))
    if key not in _PROGRAM_CACHE:
        nc = _build_program(meta)
        _PROGRAM_CACHE[key] = (nc, _CachedRunner(nc, NCORES))
    nc, runner = _PROGRAM_CACHE[key]

    s16, s32 = TOT16 // NCORES, TOT32 // NCORES
    in_maps = []
    for c in range(NCORES):
        m = dict(per_core[c])
        m["wsh16"] = blob16[c * s16:(c + 1) * s16].reshape(1, s16)
        m["wsh32"] = blob32[c * s32:(c + 1) * s32].reshape(1, s32)
        in_maps.append(m)
    return runner, runner.stack(in_maps)


def kernel(**inputs) -> np.ndarray:
    runner, stacked = _prepare(inputs)
    res = runner.run_stacked(stacked)
    out = np.concatenate([res[c]["y"][0] for c in range(NCORES)])
    return (out + np.asarray(inputs["dec_b3"], np.float32)[0]).astype(np.float32)
